# revision 2
# baseline (speedup 1.0000x reference)
"""Trainium2 Bass kernel for nn_Encoder (dense MLP with stochastic ternarization).

y = tanh(x @ (s1*T(w1,n1)) + b1) @ (s2*T(w2,n2)) + b2,  T(w,n) = (w-n>1) - (w-n<-1)

Sharding: tensor-parallel over the 16384 hidden dim across 8 cores. Each core
gets a 2048-wide hidden shard of w1/noise1/s1/b1 (column-sharded) and the
matching 2048-row shard of w2/noise2; x is replicated (host pre-transposed to
bf16). Each core computes partial yT = (h_shard @ w2_shard).T in bf16 per
512-batch block; per-block, per-256-row quarter ReduceScatters(add) hand core c
rows {256q+32c .. +32} where s2/b2 are applied. The host reassembles.

v2 structure (baseline was ~650us):
- Supply is column-major at 512KB granularity: for hidden column c (128 wide),
  pieces p=0..2 each cover 8 k-tiles. PE chains consume columns in the same
  order, so the first matmul issues ~7us in and the PE never head-of-line
  blocks on supply (supply runs ~2.4x faster than PE consumption).
- Ternarize: q = w - n (DVE), tanh(2^30(q-1)) + tanh(2^30(q+1)) (ACT, fp8 out),
  DVE add -> t2g in fp8 ({-2,0,+2} exact; factor 2 folded into s1/s2 on host).
  ACT program order keeps tern ~5 columns ahead of the PSUM->h activations.
- Phase 2 runs from resident weights: L1(b2), L2(b0), L2(b1), L1(b3), L2(b2),
  L2(b3); x2/x3 stream on the gpsimd (SWDGE) ring into recycled x slots.
- Layer-2 partials are cast to bf16 and reduced with 16 quarter-sized
  ReduceScatters (256KB in / 32KB out each) pipelined against the matmuls, so
  the post-last-matmul tail is one small RS + scale instead of ~70us.
"""

import sys

for _p in ("/opt/trn_rl_repo",):
    if _p not in sys.path:
        sys.path.insert(0, _p)

import numpy as np
import ml_dtypes

import concourse.bass as bass
import concourse.bacc as bacc
import concourse.mybir as mybir
import concourse.tile as tile
from concourse.bass_utils import run_bass_kernel_spmd

BF16 = mybir.dt.bfloat16
F32 = mybir.dt.float32
FP8 = mybir.dt.float8e4
NPBF16 = ml_dtypes.bfloat16

N_CORES = 8
B = 2048
DIN = 3072
DHID = 16384
DOUT = 1024
HSH = DHID // N_CORES    # 2048
K1 = DIN // 128          # 24 contraction tiles, layer 1
NP1 = 3                  # supply pieces per column (8 k-tiles each)
KP = K1 // NP1           # 8 k-tiles per piece
K2 = HSH // 128          # 16 contraction tiles, layer 2
NB = B // 512            # 4 batch blocks
MT = HSH // 128          # 16 hidden columns (128 wide)
ND = DOUT // 128         # 8 dout tiles
NQR = 4                  # RS quarters per block (256 dout rows each)
QROWS = DOUT // NQR      # 256
CROWS = QROWS // N_CORES  # 32 rows per core per quarter

BIGK = float(2 ** 30)

TANH = mybir.ActivationFunctionType.Tanh
MULT = mybir.AluOpType.mult
ADD = mybir.AluOpType.add


def build_bass():
    nc = bacc.Bacc("TRN2", target_bir_lowering=False, debug=False, num_devices=N_CORES)

    # x: [block, piece, 128 part(k-in), 8 k, 512 batch] bf16
    xtb = nc.dram_tensor("xtb", [NB, NP1, 128, KP, 512], BF16, kind="ExternalInput")
    # w1/n1: [col, piece, 128 part(k-in), 8 k, 128 m] f32
    w1p = nc.dram_tensor("w1p", [MT, NP1, 128, KP, 128], F32, kind="ExternalInput")
    n1p = nc.dram_tensor("n1p", [MT, NP1, 128, KP, 128], F32, kind="ExternalInput")
    s1h = nc.dram_tensor("s1h", [128, MT], F32, kind="ExternalInput")
    b1m = nc.dram_tensor("b1m", [128, MT], F32, kind="ExternalInput")
    # w2/n2: [k2, 128 part(hid-in), 8 dtile, 128 dout] f32
    w2p = nc.dram_tensor("w2p", [K2, 128, KP, 128], F32, kind="ExternalInput")
    n2p = nc.dram_tensor("n2p", [K2, 128, KP, 128], F32, kind="ExternalInput")
    s2c = nc.dram_tensor("s2c", [128, 1], F32, kind="ExternalInput")
    b2c = nc.dram_tensor("b2c", [128, 1], F32, kind="ExternalInput")

    # partition p of yTc = global dout row 256*(p//32) + 32*core + (p%32)
    yTc = nc.dram_tensor("yTc", [128, B], F32, kind="ExternalOutput")

    with tile.TileContext(nc) as tc:
        with (
            tc.tile_pool(name="const", bufs=1) as cpool,
            tc.tile_pool(name="dram", bufs=1, space="DRAM") as dpool,
            tc.tile_pool(name="t2w1", bufs=MT) as t2pool,
            tc.tile_pool(name="t2w2", bufs=1) as t22pool,
            tc.tile_pool(name="stage", bufs=2) as spool,
            tc.tile_pool(name="xtn", bufs=7) as xpool,
            tc.tile_pool(name="hblk", bufs=3 * MT) as hpool,
            tc.tile_pool(name="yblk", bufs=4) as ypool,
            tc.tile_pool(name="fin", bufs=2) as fpool,
            tc.tile_pool(name="ps1", bufs=4, space="PSUM") as pspool,
            tc.tile_pool(name="ps2", bufs=3, space="PSUM") as ps2pool,
        ):
            s1_sb = cpool.tile([128, MT], F32, tag="s1")
            b1_sb = cpool.tile([128, MT], F32, tag="b1")
            s2_sb = cpool.tile([128, 1], F32, tag="s2")
            b2_sb = cpool.tile([128, 1], F32, tag="b2")
            nc.sync.dma_start(s1_sb[:], s1h[:, :])
            nc.sync.dma_start(b1_sb[:], b1m[:, :])
            nc.sync.dma_start(s2_sb[:], s2c[:, :])
            nc.sync.dma_start(b2_sb[:], b2c[:, :])
            kneg = cpool.tile([128, 1], F32, tag="kneg")
            nc.vector.memset(kneg[:], -BIGK)
            kpos = cpool.tile([128, 1], F32, tag="kpos")
            nc.vector.memset(kpos[:], BIGK)

            # per (block, quarter) partial / scattered buffers (bf16)
            yT_q = [[dpool.tile([QROWS, 512], BF16, tag=f"yTp{b}{q}",
                                name=f"yT_b{b}q{q}") for q in range(NQR)]
                    for b in range(NB)]
            rs_q = [[dpool.tile([CROWS, 512], BF16, tag=f"rs{b}{q}",
                                name=f"rs_b{b}q{q}") for q in range(NQR)]
                    for b in range(NB)]

            # x sub-tiles: [128, 8, 512] bf16, pool cycles 7 slots over 12 loads
            xs = {}
            for b in (0, 1):
                for p in range(NP1):
                    t = xpool.tile([128, KP, 512], BF16, tag="xs",
                                   name=f"xs{b}_{p}")
                    nc.scalar.dma_start(t[:], xtb[b, p])
                    xs[(b, p)] = t

            t2g = [t2pool.tile([128, K1, 128], FP8, tag="t2", name=f"t2c{c}")
                   for c in range(MT)]
            t22 = t22pool.tile([128, K2, KP, 128], FP8, tag="t22")

            # ---- supply DMAs (sync ring, in consumption order) ----
            w_stage = []
            for c in range(MT):
                for p in range(NP1):
                    w_t = spool.tile([128, KP, 128], F32, tag="w",
                                     name=f"w1s_{c}_{p}")
                    nc.sync.dma_start(w_t[:], w1p[c, p])
                    n_t = spool.tile([128, KP, 128], F32, tag="n",
                                     name=f"n1s_{c}_{p}")
                    nc.sync.dma_start(n_t[:], n1p[c, p])
                    w_stage.append((w_t, n_t))
            # x2/x3 on the gpsimd (SWDGE) ring: independent of sync/scalar rings
            for b in (2, 3):
                for p in range(NP1):
                    t = xpool.tile([128, KP, 512], BF16, tag="xs",
                                   name=f"xs{b}_{p}")
                    nc.gpsimd.dma_start(t[:], xtb[b, p])
                    xs[(b, p)] = t
            w2_stage = []
            for k2 in range(K2):
                w_t = spool.tile([128, KP, 128], F32, tag="w", name=f"w2s_{k2}")
                nc.sync.dma_start(w_t[:], w2p[k2])
                n_t = spool.tile([128, KP, 128], F32, tag="n", name=f"n2s_{k2}")
                nc.sync.dma_start(n_t[:], n2p[k2])
                w2_stage.append((w_t, n_t))

            # ---- ternarize steps (DVE + ACT); ACT program order interleaves
            # the h activations a few columns behind the tern supply ----
            def tern_piece(dst_ap, w_t, n_t, name):
                nc.vector.tensor_sub(w_t[:], w_t[:], n_t[:])
                a1 = spool.tile([128, KP, 128], FP8, tag="a1", name=f"a1_{name}")
                nc.scalar.activation(a1[:], w_t[:], TANH, bias=kneg[:, 0:1],
                                     scale=BIGK)
                a2 = spool.tile([128, KP, 128], FP8, tag="a2", name=f"a2_{name}")
                nc.scalar.activation(a2[:], w_t[:], TANH, bias=kpos[:, 0:1],
                                     scale=BIGK)
                nc.vector.tensor_add(dst_ap, a1[:], a2[:])

            h_tiles = {}

            def h_act(b, c, ps):
                h_t = hpool.tile([128, 512], BF16, tag="h", name=f"h{b}_{c}")
                nc.scalar.activation(h_t[:], ps[:], TANH,
                                     bias=b1_sb[:, c:c + 1],
                                     scale=s1_sb[:, c:c + 1])
                h_tiles[(b, c)] = h_t

            # layer-1 chain for one (block, column) -> PSUM (not yet activated)
            def l1_chain(b, c):
                ps = pspool.tile([128, 512], F32, tag="ps", name=f"ps{b}_{c}")
                for k in range(K1):
                    nc.tensor.matmul(
                        ps[:],
                        t2g[c][:, k, :],
                        xs[(b, k // KP)][:, k % KP, :],
                        start=(k == 0), stop=(k == K1 - 1))
                return ps

            # tern for column c (3 pieces)
            def tern_col(c):
                for p in range(NP1):
                    w_t, n_t = w_stage[c * NP1 + p]
                    tern_piece(t2g[c][:, p * KP:(p + 1) * KP, :], w_t, n_t,
                               f"c{c}p{p}")

            TERN_LEAD = 5
            # phase 1: blocks 0/1 column-by-column; tern issued TERN_LEAD
            # columns ahead of the h activations on the ACT queue
            for c in range(TERN_LEAD):
                tern_col(c)
            for c in range(MT):
                if c + TERN_LEAD < MT:
                    tern_col(c + TERN_LEAD)
                elif c + TERN_LEAD == MT:
                    for k2 in range(K2):
                        w_t, n_t = w2_stage[k2]
                        tern_piece(t22[:, k2, :, :], w_t, n_t, f"w2k{k2}")
                for b in (0, 1):
                    ps = l1_chain(b, c)
                    h_act(b, c, ps)

            # layer-2 for one block: 8 dout chains, bf16 partials, quarter-RS
            def layer2_block(b):
                for d in range(ND):
                    p2 = ps2pool.tile([128, 512], F32, tag="ps2",
                                      name=f"ps2_{b}_{d}")
                    for k2 in range(K2):
                        nc.tensor.matmul(p2[:], t22[:, k2, d, :],
                                         h_tiles[(b, k2)][:],
                                         start=(k2 == 0), stop=(k2 == K2 - 1))
                    y_sb = ypool.tile([128, 512], BF16, tag="y",
                                      name=f"y{b}_{d}")
                    nc.vector.tensor_copy(y_sb[:], p2[:])
                    nc.sync.dma_start(
                        yT_q[b][d // 2][(d % 2) * 128:(d % 2 + 1) * 128, :],
                        y_sb[:],
                    )
                    if d % 2 == 1:
                        q = d // 2
                        nc.gpsimd.collective_compute(
                            "ReduceScatter",
                            mybir.AluOpType.add,
                            replica_groups=[list(range(N_CORES))],
                            ins=[yT_q[b][q].opt()],
                            outs=[rs_q[b][q].opt()],
                        )

            def fin_block(b):
                rs_sb = fpool.tile([128, 512], BF16, tag="rsb", name=f"rsb{b}")
                for q in range(NQR):
                    nc.sync.dma_start(rs_sb[q * CROWS:(q + 1) * CROWS, :],
                                      rs_q[b][q][:, :])
                out_sb = fpool.tile([128, 512], F32, tag="osb", name=f"osb{b}")
                nc.vector.tensor_scalar(
                    out_sb[:], rs_sb[:], s2_sb[:, 0:1], b2_sb[:, 0:1], MULT, ADD,
                )
                nc.sync.dma_start(yTc[:, b * 512:(b + 1) * 512], out_sb[:])

            # phase 2: resident weights; order spreads RS traffic and frees
            # h/x buffers just in time
            for c in range(MT):
                ps = l1_chain(2, c)
                h_act(2, c, ps)
            layer2_block(0)
            fin_block(0)
            layer2_block(1)
            for c in range(MT):
                ps = l1_chain(3, c)
                h_act(3, c, ps)
            fin_block(1)
            layer2_block(2)
            fin_block(2)
            layer2_block(3)
            fin_block(3)

    nc.compile()
    return nc


_NC_CACHE = {}


def _get_nc():
    if "nc" not in _NC_CACHE:
        _NC_CACHE["nc"] = build_bass()
    return _NC_CACHE["nc"]


def _make_in_maps(x, w1, s1, b1, w2, s2, b2, noise1, noise2):
    x = np.asarray(x, dtype=np.float32)
    w1 = np.asarray(w1, dtype=np.float32)
    s1 = np.asarray(s1, dtype=np.float32)
    b1 = np.asarray(b1, dtype=np.float32)
    w2 = np.asarray(w2, dtype=np.float32)
    s2 = np.asarray(s2, dtype=np.float32)
    b2 = np.asarray(b2, dtype=np.float32)
    noise1 = np.asarray(noise1, dtype=np.float32)
    noise2 = np.asarray(noise2, dtype=np.float32)

    xT = x.T.astype(NPBF16)  # [3072, 2048]
    # -> [NB, NP1, 128, KP, 512]
    xtb = np.ascontiguousarray(
        xT.reshape(NP1, KP, 128, NB, 512).transpose(3, 0, 2, 1, 4))

    def w1_tile(w):   # [din, HSH] -> [MT, NP1, 128, KP, 128]
        return np.ascontiguousarray(
            w.reshape(NP1, KP, 128, MT, 128).transpose(3, 0, 2, 1, 4))

    def w2_tile(w):   # [HSH, DOUT] -> [K2, 128, KP, 128]
        return np.ascontiguousarray(w.reshape(K2, 128, KP, 128))

    # core c, partition p -> global dout row 256*(p//32) + 32*c + (p%32)
    rows_per_core = []
    for c in range(N_CORES):
        rows = np.concatenate([
            np.arange(QROWS * q + CROWS * c, QROWS * q + CROWS * (c + 1))
            for q in range(NQR)])
        rows_per_core.append(rows)

    in_maps = []
    for c in range(N_CORES):
        hs = slice(c * HSH, (c + 1) * HSH)
        rows = rows_per_core[c]
        in_maps.append({
            "xtb": xtb,
            "w1p": w1_tile(w1[:, hs]),
            "n1p": w1_tile(noise1[:, hs]),
            "s1h": np.ascontiguousarray((0.5 * s1[hs]).reshape(MT, 128).T),
            "b1m": np.ascontiguousarray(b1[hs].reshape(MT, 128).T),
            "w2p": w2_tile(np.ascontiguousarray(w2[hs, :])),
            "n2p": w2_tile(np.ascontiguousarray(noise2[hs, :])),
            "s2c": np.ascontiguousarray((0.5 * s2[rows]).reshape(128, 1)),
            "b2c": np.ascontiguousarray(b2[rows].reshape(128, 1)),
        })
    return in_maps, rows_per_core


def kernel(x, w1, s1, b1, w2, s2, b2, noise1, noise2, _bench_out=None):
    """Full-input, full-output entry point. Shards across 8 NeuronCores."""
    nc = _get_nc()
    in_maps, rows_per_core = _make_in_maps(
        x, w1, s1, b1, w2, s2, b2, noise1, noise2)
    res = run_bass_kernel_spmd(nc, in_maps, core_ids=list(range(N_CORES)))
    if _bench_out is not None:
        _bench_out.append(res)
    yT = np.empty((DOUT, B), dtype=np.float32)
    for c in range(N_CORES):
        yT[rows_per_core[c], :] = res.results[c]["yTc"]
    return np.ascontiguousarray(yT.T).astype(np.float32)


if __name__ == "__main__":
    nc = build_bass()
    print("built OK")


# revision 6
# speedup vs baseline: 1.0598x; 1.0598x over previous
"""Trainium2 Bass kernel for nn_Encoder (dense MLP with stochastic ternarization).

y = tanh(x @ (s1*T(w1,n1)) + b1) @ (s2*T(w2,n2)) + b2,  T(w,n) = (w-n>1) - (w-n<-1)

Sharding: tensor-parallel over the 16384 hidden dim across 8 cores. Each core
gets a 2048-wide hidden shard of w1/noise1/s1/b1 (column-sharded) and the
matching 2048-row shard of w2/noise2; x is replicated (host pre-transposed to
bf16). Each core computes partial yT = (h_shard @ w2_shard).T in bf16 per
512-batch block; per-block, per-256-row quarter ReduceScatters(add) hand core c
rows {256q+32c .. +32} where s2/b2 are applied. The host reassembles.

v2 structure (baseline was ~650us):
- Supply is column-major at 512KB granularity: for hidden column c (128 wide),
  pieces p=0..2 each cover 8 k-tiles. PE chains consume columns in the same
  order, so the first matmul issues ~7us in and the PE never head-of-line
  blocks on supply (supply runs ~2.4x faster than PE consumption).
- Ternarize: q = w - n (DVE), tanh(2^30(q-1)) + tanh(2^30(q+1)) (ACT, fp8 out),
  DVE add -> t2g in fp8 ({-2,0,+2} exact; factor 2 folded into s1/s2 on host).
  ACT program order keeps tern ~5 columns ahead of the PSUM->h activations.
- Phase 2 runs from resident weights: L1(b2), L2(b0), L2(b1), L1(b3), L2(b2),
  L2(b3); x2/x3 stream on the gpsimd (SWDGE) ring into recycled x slots.
- Layer-2 partials are cast to bf16 and reduced with 16 quarter-sized
  ReduceScatters (256KB in / 32KB out each) pipelined against the matmuls, so
  the post-last-matmul tail is one small RS + scale instead of ~70us.
"""

import sys

for _p in ("/opt/trn_rl_repo",):
    if _p not in sys.path:
        sys.path.insert(0, _p)

import numpy as np
import ml_dtypes

import concourse.bass as bass
import concourse.bacc as bacc
import concourse.mybir as mybir
import concourse.tile as tile
from concourse.bass_utils import run_bass_kernel_spmd

BF16 = mybir.dt.bfloat16
F32 = mybir.dt.float32
FP8 = mybir.dt.float8e4
NPBF16 = ml_dtypes.bfloat16

N_CORES = 8
B = 2048
DIN = 3072
DHID = 16384
DOUT = 1024
HSH = DHID // N_CORES    # 2048
K1 = DIN // 128          # 24 contraction tiles, layer 1
NP1 = 3                  # supply pieces per column (8 k-tiles each)
KP = K1 // NP1           # 8 k-tiles per piece
K2 = HSH // 128          # 16 contraction tiles, layer 2
NB = B // 512            # 4 batch blocks
MT = HSH // 128          # 16 hidden columns (128 wide)
ND = DOUT // 128         # 8 dout tiles
NQR = 2                  # RS halves per block (512 dout rows each)
QROWS = DOUT // NQR      # 512
CROWS = QROWS // N_CORES  # 64 rows per core per half

BIGK = float(2 ** 30)

TANH = mybir.ActivationFunctionType.Tanh
MULT = mybir.AluOpType.mult
ADD = mybir.AluOpType.add


def build_bass():
    nc = bacc.Bacc("TRN2", target_bir_lowering=False, debug=False, num_devices=N_CORES)

    # x: [block, piece, 128 part(k-in), 8 k, 512 batch] bf16
    xtb = nc.dram_tensor("xtb", [NB, NP1, 128, KP, 512], BF16, kind="ExternalInput")
    # w1/n1: [col, piece, 128 part(k-in), 8 k, 128 m] f32
    w1p = nc.dram_tensor("w1p", [MT, NP1, 128, KP, 128], F32, kind="ExternalInput")
    n1p = nc.dram_tensor("n1p", [MT, NP1, 128, KP, 128], F32, kind="ExternalInput")
    s1h = nc.dram_tensor("s1h", [128, MT], F32, kind="ExternalInput")
    b1m = nc.dram_tensor("b1m", [128, MT], F32, kind="ExternalInput")
    # w2/n2: [k2, 128 part(hid-in), 8 dtile, 128 dout] f32
    w2p = nc.dram_tensor("w2p", [K2, 128, KP, 128], F32, kind="ExternalInput")
    n2p = nc.dram_tensor("n2p", [K2, 128, KP, 128], F32, kind="ExternalInput")
    s2c = nc.dram_tensor("s2c", [128, 1], F32, kind="ExternalInput")
    b2c = nc.dram_tensor("b2c", [128, 1], F32, kind="ExternalInput")

    # partition p of yTc = global dout row 256*(p//32) + 32*core + (p%32)
    yTc = nc.dram_tensor("yTc", [128, B], F32, kind="ExternalOutput")

    with tile.TileContext(nc) as tc:
        with (
            tc.tile_pool(name="const", bufs=1) as cpool,
            tc.tile_pool(name="dram", bufs=1, space="DRAM") as dpool,
            tc.tile_pool(name="t2w1", bufs=MT) as t2pool,
            tc.tile_pool(name="t2w2", bufs=1) as t22pool,
            tc.tile_pool(name="stage", bufs=2) as spool,
            tc.tile_pool(name="xtn", bufs=7) as xpool,
            tc.tile_pool(name="hblk", bufs=3 * MT) as hpool,
            tc.tile_pool(name="yblk", bufs=4) as ypool,
            tc.tile_pool(name="fin", bufs=2) as fpool,
            tc.tile_pool(name="ps1", bufs=4, space="PSUM") as pspool,
            tc.tile_pool(name="ps2", bufs=3, space="PSUM") as ps2pool,
        ):
            kneg = cpool.tile([128, 1], F32, tag="kneg")
            nc.vector.memset(kneg[:], -BIGK)
            kpos = cpool.tile([128, 1], F32, tag="kpos")
            nc.vector.memset(kpos[:], BIGK)

            # per (block, quarter) partial / scattered buffers (bf16)
            yT_q = [[dpool.tile([QROWS, 512], BF16, tag=f"yTp{b}{q}",
                                name=f"yT_b{b}q{q}") for q in range(NQR)]
                    for b in range(NB)]
            rs_q = [[dpool.tile([CROWS, 512], BF16, tag=f"rs{b}{q}",
                                name=f"rs_b{b}q{q}") for q in range(NQR)]
                    for b in range(NB)]

            # x sub-tiles: [128, 8, 512] bf16, pool cycles 7 slots over 12 loads
            xs = {}

            def x_load(b, p, engine):
                t = xpool.tile([128, KP, 512], BF16, tag="xs",
                               name=f"xs{b}_{p}")
                engine.dma_start(t[:], xtb[b, p])
                xs[(b, p)] = t

            t2g = [t2pool.tile([128, K1, 128], FP8, tag="t2", name=f"t2c{c}")
                   for c in range(MT)]
            t22 = t22pool.tile([128, K2, KP, 128], FP8, tag="t22")

            # ---- supply DMAs (sync ring, in consumption order) ----
            # x0p0 first so the very first chain can start; x0p1/p2 ride the
            # scalar ring; x1 slots into the sync stream after column 5.
            x_load(0, 0, nc.sync)
            x_load(0, 1, nc.scalar)
            x_load(0, 2, nc.scalar)

            w_stage = []

            def w1_load(c):
                for p in range(NP1):
                    w_t = spool.tile([128, KP, 128], F32, tag="w",
                                     name=f"w1s_{c}_{p}")
                    nc.sync.dma_start(w_t[:], w1p[c, p])
                    n_t = spool.tile([128, KP, 128], F32, tag="n",
                                     name=f"n1s_{c}_{p}")
                    nc.sync.dma_start(n_t[:], n1p[c, p])
                    w_stage.append((w_t, n_t))

            s1_sb = cpool.tile([128, MT], F32, tag="s1")
            b1_sb = cpool.tile([128, MT], F32, tag="b1")
            s2_sb = cpool.tile([128, 1], F32, tag="s2")
            b2_sb = cpool.tile([128, 1], F32, tag="b2")

            for c in range(MT):
                w1_load(c)
                if c == 1:
                    nc.sync.dma_start(s1_sb[:], s1h[:, :])
                    nc.sync.dma_start(b1_sb[:], b1m[:, :])
                    nc.sync.dma_start(s2_sb[:], s2c[:, :])
                    nc.sync.dma_start(b2_sb[:], b2c[:, :])
                if c == 5:
                    for p in range(NP1):
                        x_load(1, p, nc.sync)
            # x2/x3 on the gpsimd (SWDGE) ring: independent of sync/scalar rings
            for b in (2, 3):
                for p in range(NP1):
                    x_load(b, p, nc.gpsimd)
            w2_stage = []
            for k2 in range(K2):
                w_t = spool.tile([128, KP, 128], F32, tag="w", name=f"w2s_{k2}")
                nc.sync.dma_start(w_t[:], w2p[k2])
                n_t = spool.tile([128, KP, 128], F32, tag="n", name=f"n2s_{k2}")
                nc.sync.dma_start(n_t[:], n2p[k2])
                w2_stage.append((w_t, n_t))

            # ---- ternarize steps (DVE + ACT); ACT program order interleaves
            # the h activations a few columns behind the tern supply ----
            def tern_piece(dst_ap, w_t, n_t, name):
                nc.vector.tensor_sub(w_t[:], w_t[:], n_t[:])
                a1 = spool.tile([128, KP, 128], FP8, tag="a1", name=f"a1_{name}")
                nc.scalar.activation(a1[:], w_t[:], TANH, bias=kneg[:, 0:1],
                                     scale=BIGK)
                a2 = spool.tile([128, KP, 128], FP8, tag="a2", name=f"a2_{name}")
                nc.scalar.activation(a2[:], w_t[:], TANH, bias=kpos[:, 0:1],
                                     scale=BIGK)
                nc.vector.tensor_add(dst_ap, a1[:], a2[:])

            h_tiles = {}

            def h_act(b, c, ps):
                h_t = hpool.tile([128, 512], BF16, tag="h", name=f"h{b}_{c}")
                nc.scalar.activation(h_t[:], ps[:], TANH,
                                     bias=b1_sb[:, c:c + 1],
                                     scale=s1_sb[:, c:c + 1])
                h_tiles[(b, c)] = h_t

            # layer-1 chain for one (block, column) -> PSUM (not yet activated)
            def l1_chain(b, c):
                ps = pspool.tile([128, 512], F32, tag="ps", name=f"ps{b}_{c}")
                for k in range(K1):
                    nc.tensor.matmul(
                        ps[:],
                        t2g[c][:, k, :],
                        xs[(b, k // KP)][:, k % KP, :],
                        start=(k == 0), stop=(k == K1 - 1))
                return ps

            # tern for column c (3 pieces)
            def tern_col(c):
                for p in range(NP1):
                    w_t, n_t = w_stage[c * NP1 + p]
                    tern_piece(t2g[c][:, p * KP:(p + 1) * KP, :], w_t, n_t,
                               f"c{c}p{p}")

            TERN_LEAD = 5
            # phase 1: blocks 0/1 column-by-column; tern issued TERN_LEAD
            # columns ahead of the h activations on the ACT queue
            for c in range(TERN_LEAD):
                tern_col(c)
            for c in range(MT):
                if c + TERN_LEAD < MT:
                    tern_col(c + TERN_LEAD)
                elif c + TERN_LEAD == MT:
                    for k2 in range(K2):
                        w_t, n_t = w2_stage[k2]
                        tern_piece(t22[:, k2, :, :], w_t, n_t, f"w2k{k2}")
                for b in (0, 1):
                    ps = l1_chain(b, c)
                    h_act(b, c, ps)

            # layer-2 for one block: 8 dout chains, bf16 partials, quarter-RS
            def layer2_block(b):
                for d in range(ND):
                    p2 = ps2pool.tile([128, 512], F32, tag="ps2",
                                      name=f"ps2_{b}_{d}")
                    for k2 in range(K2):
                        nc.tensor.matmul(p2[:], t22[:, k2, d, :],
                                         h_tiles[(b, k2)][:],
                                         start=(k2 == 0), stop=(k2 == K2 - 1))
                    y_sb = ypool.tile([128, 512], BF16, tag="y",
                                      name=f"y{b}_{d}")
                    nc.vector.tensor_copy(y_sb[:], p2[:])
                    nc.sync.dma_start(
                        yT_q[b][d // 4][(d % 4) * 128:(d % 4 + 1) * 128, :],
                        y_sb[:],
                    )
                    if d % 4 == 3:
                        q = d // 4
                        nc.gpsimd.collective_compute(
                            "ReduceScatter",
                            mybir.AluOpType.add,
                            replica_groups=[list(range(N_CORES))],
                            ins=[yT_q[b][q].opt()],
                            outs=[rs_q[b][q].opt()],
                        )

            def fin_block(b):
                rs_sb = fpool.tile([128, 512], BF16, tag="rsb", name=f"rsb{b}")
                for q in range(NQR):
                    nc.sync.dma_start(rs_sb[q * CROWS:(q + 1) * CROWS, :],
                                      rs_q[b][q][:, :])
                out_sb = fpool.tile([128, 512], F32, tag="osb", name=f"osb{b}")
                nc.vector.tensor_scalar(
                    out_sb[:], rs_sb[:], s2_sb[:, 0:1], b2_sb[:, 0:1], MULT, ADD,
                )
                nc.sync.dma_start(yTc[:, b * 512:(b + 1) * 512], out_sb[:])

            # phase 2: resident weights; order spreads RS traffic and frees
            # h/x buffers just in time
            for c in range(MT):
                ps = l1_chain(2, c)
                h_act(2, c, ps)
            layer2_block(0)
            fin_block(0)
            layer2_block(1)
            for c in range(MT):
                ps = l1_chain(3, c)
                h_act(3, c, ps)
            fin_block(1)
            layer2_block(2)
            fin_block(2)
            layer2_block(3)
            fin_block(3)

    nc.compile()
    return nc


_NC_CACHE = {}


def _get_nc():
    if "nc" not in _NC_CACHE:
        _NC_CACHE["nc"] = build_bass()
    return _NC_CACHE["nc"]


def _make_in_maps(x, w1, s1, b1, w2, s2, b2, noise1, noise2):
    x = np.asarray(x, dtype=np.float32)
    w1 = np.asarray(w1, dtype=np.float32)
    s1 = np.asarray(s1, dtype=np.float32)
    b1 = np.asarray(b1, dtype=np.float32)
    w2 = np.asarray(w2, dtype=np.float32)
    s2 = np.asarray(s2, dtype=np.float32)
    b2 = np.asarray(b2, dtype=np.float32)
    noise1 = np.asarray(noise1, dtype=np.float32)
    noise2 = np.asarray(noise2, dtype=np.float32)

    xT = x.T.astype(NPBF16)  # [3072, 2048]
    # -> [NB, NP1, 128, KP, 512]
    xtb = np.ascontiguousarray(
        xT.reshape(NP1, KP, 128, NB, 512).transpose(3, 0, 2, 1, 4))

    def w1_tile(w):   # [din, HSH] -> [MT, NP1, 128, KP, 128]
        return np.ascontiguousarray(
            w.reshape(NP1, KP, 128, MT, 128).transpose(3, 0, 2, 1, 4))

    def w2_tile(w):   # [HSH, DOUT] -> [K2, 128, KP, 128]
        return np.ascontiguousarray(w.reshape(K2, 128, KP, 128))

    # core c, partition p -> global dout row 256*(p//32) + 32*c + (p%32)
    rows_per_core = []
    for c in range(N_CORES):
        rows = np.concatenate([
            np.arange(QROWS * q + CROWS * c, QROWS * q + CROWS * (c + 1))
            for q in range(NQR)])
        rows_per_core.append(rows)

    in_maps = []
    for c in range(N_CORES):
        hs = slice(c * HSH, (c + 1) * HSH)
        rows = rows_per_core[c]
        in_maps.append({
            "xtb": xtb,
            "w1p": w1_tile(w1[:, hs]),
            "n1p": w1_tile(noise1[:, hs]),
            "s1h": np.ascontiguousarray((0.5 * s1[hs]).reshape(MT, 128).T),
            "b1m": np.ascontiguousarray(b1[hs].reshape(MT, 128).T),
            "w2p": w2_tile(np.ascontiguousarray(w2[hs, :])),
            "n2p": w2_tile(np.ascontiguousarray(noise2[hs, :])),
            "s2c": np.ascontiguousarray((0.5 * s2[rows]).reshape(128, 1)),
            "b2c": np.ascontiguousarray(b2[rows].reshape(128, 1)),
        })
    return in_maps, rows_per_core


def kernel(x, w1, s1, b1, w2, s2, b2, noise1, noise2, _bench_out=None):
    """Full-input, full-output entry point. Shards across 8 NeuronCores."""
    nc = _get_nc()
    in_maps, rows_per_core = _make_in_maps(
        x, w1, s1, b1, w2, s2, b2, noise1, noise2)
    res = run_bass_kernel_spmd(nc, in_maps, core_ids=list(range(N_CORES)))
    if _bench_out is not None:
        _bench_out.append(res)
    yT = np.empty((DOUT, B), dtype=np.float32)
    for c in range(N_CORES):
        yT[rows_per_core[c], :] = res.results[c]["yTc"]
    return np.ascontiguousarray(yT.T).astype(np.float32)


if __name__ == "__main__":
    nc = build_bass()
    print("built OK")
